# revision 1
# baseline (speedup 1.0000x reference)
"""Trainium2 Bass kernel for a 6-layer post-BatchNorm transformer encoder.

Reference model:
  x = emb[seq] + pes                                  # [B,S,D] = [4,512,1024]
  6x: x = BN(x + attn(x)); x = BN(x + ffn(x))
  BN = per-channel batch stats over (B,S), eps=1e-3.

Sharding: token-sharded data parallel across 8 NeuronCores. Core c owns the
256 contiguous tokens [256c, 256c+256) = batch c//2, sequence half c%2. All
matmuls are local full-width (every core streams the full bf16 weights from
HBM in 1MB chunks through a ring); residual adds and BatchNorm application
are local. Cross-core communication per layer:
  - one pair AllGather (cores 2b,2b+1) of K^T and V (bf16, 1MB in / 2MB out)
    so attention sees the full 512-key sequence of its batch,
  - two 8KB AllGathers of per-core BN partial sums/sumsq (the only global
    coupling BatchNorm actually needs).
bk/bv/bo/b2 biases cancel mathematically (bk/bv through softmax rows summing
to 1, bo/b2 inside BN mean subtraction) and are dropped; bq and b1 are kept.

Numerics: all matmuls in bf16 (fp32 PSUM accumulation); x kept in fp32
master + bf16 matmul copy; BN statistics in fp32.

Layout: activations transposed [128 part, dtile, tokens]; weights natural
[Din, Dout] serve as lhsT. Attention per head pair packs the two heads at
partition bases 0/64 (row-group concurrency on the PE). Softmax sums ride
as a ones-column in the even head's V (PSUM rows 64) and a separate
ones-row matmul into PSUM row 96 for the odd head; the odd head's U lands
at PSUM rows 64:128 via output col-group 64, so every downstream copy is
partition-aligned. Embedding gather uses dma_gather(transpose=True), which
lands rows directly in the transposed layout.
"""

import os

import numpy as np

import concourse.bass as bass
import concourse.mybir as mybir
import concourse.tile as tile
from concourse import bacc
from concourse.bass import ts

# ---------------------------------------------------------------- dims
V, D, L, H, B, S = 32000, 1024, 6, 16, 4, 512
HD = D // H            # 64
DF = 4 * D             # 4096
EPS = 1e-3
NC = 8                 # cores
T = B * S              # 2048 tokens total
TL = T // NC           # 256 tokens per core
P = 128                # partitions
DT = D // P            # 8 d-tiles
FMT = DF // P          # 32 ffn1 m-tiles
SK = S // P            # 4 key chunks per batch

f32 = mybir.dt.float32
bf16 = mybir.dt.bfloat16
i16 = mybir.dt.int16
AF = mybir.ActivationFunctionType
ALU = mybir.AluOpType

ALLGRP = [list(range(NC))]
KVGRP = [[2 * b, 2 * b + 1] for b in range(B)]

N_LAYERS = int(os.environ.get("TRN_KERNEL_LAYERS", str(L)))
DEBUG_TAPS = os.environ.get("TRN_KERNEL_DEBUG", "0") == "1"
QSPLIT = os.environ.get("TRN_QSPLIT", "1") == "1"
FASTRECIP = os.environ.get("TRN_FASTRECIP", "1") == "1"
USE_TTR = os.environ.get("TRN_TTR", "0") == "1"

KVB = TL * D           # elements of the K^T block in the kv exchange
VWB = H * (HD + 1)     # 1040: per-token V row incl ones columns
VB2 = 2 * P * VWB      # elements of the interleaved V block
KVT = VB2 + KVB        # total elements per rank in the kv exchange


def build_module(n_layers=None):
    if n_layers is None:
        n_layers = N_LAYERS
    nc = bacc.Bacc("TRN2", target_bir_lowering=False, debug=False,
                   num_devices=NC)

    dt_ = nc.dram_tensor
    io = {
        "emb": dt_("emb", [V, D], bf16, kind="ExternalInput").ap(),
        "idx": dt_("idx", [16, TL // 16], i16, kind="ExternalInput").ap(),
        "pesT": dt_("pesT", [D, TL], f32, kind="ExternalInput").ap(),
        "wq": dt_("wq", [L, D, D], bf16, kind="ExternalInput").ap(),
        "wk": dt_("wk", [L, D, D], bf16, kind="ExternalInput").ap(),
        "wv": dt_("wv", [L, D, D], bf16, kind="ExternalInput").ap(),
        "wo": dt_("wo", [L, D, D], bf16, kind="ExternalInput").ap(),
        "w1": dt_("w1", [L, D, DF], bf16, kind="ExternalInput").ap(),
        "w2": dt_("w2", [L, DF, D], bf16, kind="ExternalInput").ap(),
        "bq": dt_("bq", [L, P, DT], f32, kind="ExternalInput").ap(),
        "b1": dt_("b1", [L, P, FMT], f32, kind="ExternalInput").ap(),
        "g1": dt_("g1", [L, P, DT], f32, kind="ExternalInput").ap(),
        "be1": dt_("be1", [L, P, DT], f32, kind="ExternalInput").ap(),
        "g2": dt_("g2", [L, P, DT], f32, kind="ExternalInput").ap(),
        "be2": dt_("be2", [L, P, DT], f32, kind="ExternalInput").ap(),
        "out": dt_("out", [D, TL], f32, kind="ExternalOutput").ap(),
    }
    if DEBUG_TAPS:
        for nm, shp in [("dbg_x", [D, TL]), ("dbg_y1", [D, TL]),
                        ("dbg_x2", [D, TL]), ("dbg_y2", [D, TL])]:
            io[nm] = dt_(nm, shp, f32, kind="ExternalOutput").ap()
        for nm, shp in [("dbg_q", [D, TL]), ("dbg_k", [D, S]),
                        ("dbg_vsb", [P, SK * H * (HD + 1)]),
                        ("dbg_attnT", [D, TL]), ("dbg_h", [DF, TL])]:
            io[nm] = dt_(nm, shp, bf16, kind="ExternalOutput").ap()

    with tile.TileContext(nc) as tc:
        _build(tc, n_layers, io)
    nc.compile()
    return nc


def _build(tc, n_layers, io):
    from contextlib import ExitStack
    nc = tc.nc
    att_scale = 1.0 / np.sqrt(HD)
    dmae = nc.scalar if QSPLIT else nc.sync

    st_ = ExitStack()
    persist = st_.enter_context(tc.tile_pool(name="persist", bufs=1))
    wpool = st_.enter_context(tc.tile_pool(name="wpool", bufs=10))
    ppool = st_.enter_context(tc.tile_pool(name="ppool", bufs=2))
    epool = st_.enter_context(tc.tile_pool(name="epool", bufs=10))
    spool = st_.enter_context(tc.tile_pool(name="spool", bufs=2))
    ps = st_.enter_context(tc.tile_pool(name="ps", bufs=3, space="PSUM"))
    drin = st_.enter_context(tc.tile_pool(name="drin", bufs=2, space="DRAM"))
    drout = st_.enter_context(tc.tile_pool(name="drout", bufs=2, space="DRAM"))

    # ---------------- persistent tiles
    xf32a = persist.tile([P, DT, TL], f32, name="xf32a")
    xf32b = persist.tile([P, DT, TL], f32, name="xf32b")
    xb16 = persist.tile([P, DT, TL], bf16, name="xb16")
    qT = persist.tile([P, DT, TL], bf16, name="qT")
    kloc = persist.tile([P, DT, TL], bf16, name="kloc")
    vloc = persist.tile([P, 2, H, HD + 1], bf16, name="vloc")
    kT = persist.tile([P, DT, S], bf16, name="kT")
    vsb = persist.tile([P, SK, H, HD + 1], bf16, name="vsb")
    attnT = persist.tile([P, DT, TL], bf16, name="attnT")
    ht = persist.tile([P, FMT, TL], bf16, name="ht")
    onesb = persist.tile([P, P], bf16, name="onesb")
    idxs = persist.tile([P, TL // 16], i16, name="idxs")

    nc.vector.memset(onesb[:], 1.0)
    nc.vector.memset(vloc[:, :, :, HD:HD + 1], 1.0)
    for r_ in range(P // 16):
        nc.sync.dma_start(idxs[16 * r_:16 * (r_ + 1), :], io["idx"])

    # ---------------- embedding: x^T = (emb[seq])^T + pes^T
    pes_sb = spool.tile([P, DT, TL], f32, tag="pes", bufs=1, name="pes_sb")
    dmae.dma_start(pes_sb[:], io["pesT"].rearrange("(k p) t -> p k t", p=P))
    gt = spool.tile([P, DT, TL], bf16, tag="gt", bufs=1, name="gt")
    nc.gpsimd.dma_gather(
        out_ap=gt[:], in_ap=io["emb"], idxs_ap=idxs[:],
        num_idxs=TL, num_idxs_reg=TL, elem_size=D, transpose=True)
    for k in range(DT):
        nc.vector.tensor_tensor(out=xf32a[:, k, :], in0=gt[:, k, :],
                                in1=pes_sb[:, k, :], op=ALU.add)
    nc.vector.tensor_copy(xb16[:], xf32a[:])

    if DEBUG_TAPS:
        nc.sync.dma_start(io["dbg_x"].rearrange("(k p) t -> p k t", p=P),
                          xf32a[:])

    xcur = xf32a
    xnxt = xf32b

    # ---------------- per-layer param loads (small)
    def load_params(l):
        bq_sb = ppool.tile([P, DT], f32, tag="bq", name=f"bq{l}")
        b1_sb = ppool.tile([P, FMT], f32, tag="b1", name=f"b1{l}")
        g1_sb = ppool.tile([P, DT], f32, tag="g1", name=f"g1{l}")
        be1_sb = ppool.tile([P, DT], f32, tag="be1", name=f"be1{l}")
        g2_sb = ppool.tile([P, DT], f32, tag="g2", name=f"g2{l}")
        be2_sb = ppool.tile([P, DT], f32, tag="be2", name=f"be2{l}")
        dmae.dma_start(bq_sb[:], io["bq"][l])
        dmae.dma_start(b1_sb[:], io["b1"][l])
        dmae.dma_start(g1_sb[:], io["g1"][l])
        dmae.dma_start(be1_sb[:], io["be1"][l])
        dmae.dma_start(g2_sb[:], io["g2"][l])
        dmae.dma_start(be2_sb[:], io["be2"][l])
        return bq_sb, b1_sb, g1_sb, be1_sb, g2_sb, be2_sb

    # weight chunk loader: returns [P, DT, 512] (half the out-cols of a DxD
    # weight) or [P, 4, D] (4 k-tiles of w2)
    def wchunk(src_ap, l, nm):
        t = wpool.tile(list(src_ap.shape), bf16, tag="w", name=nm)
        nc.sync.dma_start(t[:], src_ap)
        return t

    # BN stats -> AllGather -> sc/sh
    def bn_reduce(lbl, stats, g_sb, be_sb):
        sti = drin.tile([P * 16], f32, tag="sti", name=f"sti{lbl}")
        sto = drout.tile([NC * P * 16], f32, tag="sto", addr_space="Shared",
                         name=f"sto{lbl}")
        dmae.dma_start(sti[:].rearrange("(p s) -> p s", p=P), stats[:])
        nc.gpsimd.collective_compute(
            "AllGather", ALU.bypass, replica_groups=ALLGRP,
            ins=[sti[:].opt()], outs=[sto[:].opt()])
        ld = spool.tile([P, NC, 16], f32, tag="ld", name=f"ld{lbl}")
        dmae.dma_start(ld[:], sto[:].rearrange("(r p s) -> p r s", p=P, s=16))
        u1 = spool.tile([P, 4, 16], f32, tag="u1", name=f"u1{lbl}")
        nc.vector.tensor_tensor(out=u1[:], in0=ld[:, 0:4, :], in1=ld[:, 4:8, :],
                                op=ALU.add)
        u2 = spool.tile([P, 2, 16], f32, tag="u2", name=f"u2{lbl}")
        nc.vector.tensor_tensor(out=u2[:], in0=u1[:, 0:2, :], in1=u1[:, 2:4, :],
                                op=ALU.add)
        tot = spool.tile([P, 16], f32, tag="tot", name=f"tot{lbl}")
        nc.vector.tensor_tensor(out=tot[:], in0=u2[:, 0, :], in1=u2[:, 1, :],
                                op=ALU.add)
        mean = spool.tile([P, DT], f32, tag="mean", name=f"mean{lbl}")
        nc.vector.tensor_scalar_mul(mean[:], tot[:, 0:DT], 1.0 / T)
        msq = spool.tile([P, DT], f32, tag="msq", name=f"msq{lbl}")
        nc.vector.tensor_tensor(out=msq[:], in0=mean[:], in1=mean[:], op=ALU.mult)
        veps = spool.tile([P, DT], f32, tag="veps", name=f"veps{lbl}")
        nc.vector.scalar_tensor_tensor(out=veps[:], in0=tot[:, DT:16],
                                       scalar=1.0 / T, in1=msq[:],
                                       op0=ALU.mult, op1=ALU.subtract)
        nc.vector.tensor_scalar_add(veps[:], veps[:], EPS)
        rec = spool.tile([P, DT], f32, tag="rec", name=f"rec{lbl}")
        nc.vector.reciprocal(rec[:], veps[:])
        rstd = spool.tile([P, DT], f32, tag="rstd", name=f"rstd{lbl}")
        nc.scalar.sqrt(rstd[:], rec[:])
        sc = spool.tile([P, DT], f32, tag="sc", name=f"sc{lbl}")
        nc.vector.tensor_tensor(out=sc[:], in0=g_sb[:], in1=rstd[:], op=ALU.mult)
        sh = spool.tile([P, DT], f32, tag="sh", name=f"sh{lbl}")
        nc.vector.tensor_tensor(out=sh[:], in0=mean[:], in1=sc[:], op=ALU.mult)
        nc.vector.tensor_tensor(out=sh[:], in0=be_sb[:], in1=sh[:], op=ALU.subtract)
        return sc, sh

    # ---------------- layers
    for l in range(n_layers):
        bq_sb, b1_sb, g1_sb, be1_sb, g2_sb, be2_sb = load_params(l)

        wk_r = io["wk"][l].rearrange("(k p) m -> p k m", p=P)
        wv_r = io["wv"][l].rearrange("(k p) m -> p k m", p=P)
        wq_r = io["wq"][l].rearrange("(k p) m -> p k m", p=P)
        wo_r = io["wo"][l].rearrange("(k p) m -> p k m", p=P)
        w1_r = io["w1"][l].rearrange("(k p) m -> p k m", p=P)
        w2_r = io["w2"][l].rearrange("(k p) m -> p k m", p=P)

        wk_ch = [wchunk(wk_r[:, :, ts(h, 512)], l, f"wk{l}_{h}") for h in range(2)]
        wv_ch = [wchunk(wv_r[:, :, ts(h, 512)], l, f"wv{l}_{h}") for h in range(2)]
        wq_ch = [wchunk(wq_r[:, :, ts(h, 512)], l, f"wq{l}_{h}") for h in range(2)]

        # ---- K projection (local tokens): K^T = Wk^T x^T
        for g in range(DT):
            psk = ps.tile([P, TL], f32, tag="mm", name=f"psk{l}_{g}")
            for k in range(DT):
                nc.tensor.matmul(psk[:], wk_ch[g // 4][:, k, ts(g % 4, P)],
                                 xb16[:, k, :], start=(k == 0), stop=(k == DT - 1))
            nc.vector.tensor_copy(kloc[:, g, :], psk[:])

        # ---- V projection, token-major: V = x W_v (x tiles stationary)
        for mt in range(2):
            for nb in range(4):
                psv = ps.tile([P, TL], f32, tag="mm", name=f"psv{l}_{mt}_{nb}")
                for k in range(DT):
                    nc.tensor.matmul(
                        psv[:], xb16[:, k, ts(mt, P)],
                        wv_ch[nb // 2][:, k, ts(nb % 2, 256)],
                        start=(k == 0), stop=(k == DT - 1))
                nc.vector.tensor_copy(
                    vloc[:, mt, 4 * nb:4 * nb + 4, 0:HD],
                    psv[:].rearrange("p (h x) -> p h x", h=4))

        # ---- ship local K/V, gather the batch pair's full K/V
        kvi = drin.tile([KVT], bf16, tag="kvi", name=f"kvi{l}")
        kvo = drout.tile([2 * KVT], bf16, tag="kvo", name=f"kvo{l}")
        dmae.dma_start(
            kvi[0:VB2].rearrange("(p q) -> p q", p=P),
            vloc[:].rearrange("p a h x -> p (a h x)"))
        dmae.dma_start(
            kvi[VB2:KVT].rearrange("(g p t) -> p g t", g=DT, p=P), kloc[:])
        nc.gpsimd.collective_compute(
            "AllGather", ALU.bypass, replica_groups=KVGRP,
            ins=[kvi[:].opt()], outs=[kvo[:].opt()])

        # ---- Q projection (overlaps the AllGather)
        for g in range(DT):
            psq = ps.tile([P, TL], f32, tag="mm", name=f"psq{l}_{g}")
            for k in range(DT):
                nc.tensor.matmul(psq[:], wq_ch[g // 4][:, k, ts(g % 4, P)],
                                 xb16[:, k, :], start=(k == 0), stop=(k == DT - 1))
            nc.vector.tensor_scalar_add(qT[:, g, :], psq[:], bq_sb[:, g:g + 1])

        # ---- land gathered K/V (all byte-contiguous: V ships pre-interleaved)
        for hf in range(2):
            base = hf * KVT
            dmae.dma_start(
                kT[:, :, ts(hf, TL)],
                kvo[base + VB2:base + KVT].rearrange(
                    "(g p t) -> p g t", g=DT, p=P))
            dmae.dma_start(
                vsb[:, 2 * hf:2 * hf + 2, :, :].rearrange(
                    "p a h x -> p (a h x)"),
                kvo[base:base + VB2].rearrange("(p q) -> p q", p=P))

        if DEBUG_TAPS and l == 0:
            nc.sync.dma_start(io["dbg_q"].rearrange("(k p) t -> p k t", p=P), qT[:])
            nc.sync.dma_start(io["dbg_k"].rearrange("(k p) t -> p k t", p=P), kT[:])
            nc.sync.dma_start(
                io["dbg_vsb"].rearrange("p (a b c) -> p a b c", a=SK, b=H), vsb[:])

        wo_ch = [wchunk(wo_r[:, :, ts(h, 512)], l, f"wo{l}_{h}") for h in range(2)]

        # ---- attention. Phase A: all pairs' scores + exp (exp batched over
        # 2 key-chunks); PE streams scores back-to-back while scalar exps.
        eall = []
        for g in range(DT):
            epair = [[None, None], [None, None]]
            for kcb in range(2):
                sst = [None, None]
                for tw in range(2):
                    sst[tw] = ps.tile([P, 2, TL], f32, tag="s", bufs=2,
                                      name=f"pss{l}_{g}_{kcb}_{tw}")
                for j in range(2):
                    for tw in range(2):
                        hp = 64 * tw
                        nc.tensor.matmul(
                            sst[tw][:, j, :],
                            kT[hp:hp + HD, g, ts(2 * kcb + j, P)],
                            qT[hp:hp + HD, g, :], start=True, stop=True)
                for tw in range(2):
                    et = epool.tile([P, 2, TL], bf16, tag="e", bufs=18,
                                    name=f"et{l}_{g}_{kcb}_{tw}")
                    nc.scalar.activation(et[:], sst[tw][:], AF.Exp,
                                         scale=att_scale)
                    epair[tw][kcb] = et
            eall.append(epair)

        # Phase B: per pair, U accumulation + softmax normalization. The even
        # head's sumexp rides the V|1 ones-column (PSUM row 64); the odd
        # head's comes from a ones-row matmul into row 96. One reciprocal
        # over rows 64:97 covers both (partition-parallel on the DVE).
        for g in range(DT):
            epair = eall[g]
            bankA = ps.tile([P, TL], f32, tag="u", name=f"bA{l}_{g}")
            bankB = ps.tile([P, TL], f32, tag="u", name=f"bB{l}_{g}")
            for kc in range(SK):
                fl, ll = (kc == 0), (kc == SK - 1)
                ee = epair[0][kc // 2][:, kc % 2, :]
                eo = epair[1][kc // 2][:, kc % 2, :]
                nc.tensor.matmul(bankA[0:HD + 1, :],
                                 vsb[:, kc, 2 * g, 0:HD + 1], ee,
                                 start=fl, stop=ll)
                nc.tensor.matmul(bankB[64:128, :],
                                 vsb[:, kc, 2 * g + 1, 0:HD], eo,
                                 start=fl, stop=ll)
                nc.tensor.matmul(bankA[96:97, :], onesb[:, 0:1], eo,
                                 start=fl, stop=ll, tile_position=(0, 96))
            rs = spool.tile([P, TL], bf16, tag="rs", name=f"rs{l}_{g}")
            with nc.allow_low_precision(reason="softmax 1/sumexp as bf16"):
                nc.vector.reciprocal(rs[HD:97, :], bankA[HD:97, :])
            psr = ps.tile([P, TL], f32, tag="u", name=f"psr{l}_{g}")
            nc.tensor.matmul(psr[0:64, :], onesb[HD:HD + 1, 0:64],
                             rs[HD:HD + 1, :], start=True, stop=True)
            nc.tensor.matmul(psr[64:128, :], onesb[96:97, 0:64],
                             rs[96:97, :], start=True, stop=True,
                             tile_position=(96, 64))
            usbE = epool.tile([P, TL], bf16, tag="usb", bufs=6,
                              name=f"uE{l}_{g}")
            usbO = epool.tile([P, TL], bf16, tag="usb", bufs=6,
                              name=f"uO{l}_{g}")
            nc.vector.tensor_copy(usbE[0:64, :], bankA[0:64, :])
            nc.vector.tensor_copy(usbO[64:128, :], bankB[64:128, :])
            nc.vector.tensor_tensor(out=attnT[0:64, g, :], in0=usbE[0:64, :],
                                    in1=psr[0:64, :], op=ALU.mult)
            nc.vector.tensor_tensor(out=attnT[64:128, g, :], in0=usbO[64:128, :],
                                    in1=psr[64:128, :], op=ALU.mult)

        if DEBUG_TAPS and l == 0:
            nc.sync.dma_start(
                io["dbg_attnT"].rearrange("(k p) t -> p k t", p=P), attnT[:])

        w1_ch = [wchunk(w1_r[:, :, ts(h, 512)], l, f"w1{l}_{h}") for h in range(8)]

        # ---- Wo + residual -> y1 (fp32) with fused BN partial stats
        st1 = spool.tile([P, 16], f32, tag="st", name=f"st1_{l}")
        sqs = spool.tile([P, TL], f32, tag="sqs", name=f"sq1_{l}")
        for m in range(DT):
            pso = ps.tile([P, TL], f32, tag="mm", name=f"pso{l}_{m}")
            for k in range(DT):
                nc.tensor.matmul(pso[:], wo_ch[m // 4][:, k, ts(m % 4, P)],
                                 attnT[:, k, :], start=(k == 0), stop=(k == DT - 1))
            nc.vector.scalar_tensor_tensor(
                out=xnxt[:, m, :], in0=pso[:], scalar=1.0, in1=xcur[:, m, :],
                op0=ALU.mult, op1=ALU.add, accum_out=st1[:, m:m + 1])
            if USE_TTR:
                nc.vector.tensor_tensor_reduce(
                    out=sqs[:], in0=xnxt[:, m, :], in1=xnxt[:, m, :], scale=1.0,
                    scalar=0.0, op0=ALU.mult, op1=ALU.add,
                    accum_out=st1[:, DT + m:DT + m + 1])
            else:
                nc.scalar.activation(sqs[:], xnxt[:, m, :], AF.Square,
                                     accum_out=st1[:, DT + m:DT + m + 1])

        if DEBUG_TAPS and l == 0:
            nc.sync.dma_start(io["dbg_y1"].rearrange("(k p) t -> p k t", p=P),
                              xnxt[:])

        # y1 currently lives in xnxt; BN1 normalizes it in place into
        # xcur-for-ffn (xnxt holds y1; apply writes xb16 + xnxt fp32)
        sc1, sh1 = bn_reduce(f"a{l}", st1, g1_sb, be1_sb)
        for m in range(DT):
            nc.scalar.activation(xb16[:, m, :], xnxt[:, m, :], AF.Identity,
                                 bias=sh1[:, m:m + 1], scale=sc1[:, m:m + 1])
            nc.vector.tensor_scalar(out=xnxt[:, m, :], in0=xnxt[:, m, :],
                                    scalar1=sc1[:, m:m + 1],
                                    scalar2=sh1[:, m:m + 1],
                                    op0=ALU.mult, op1=ALU.add)
        xcur, xnxt = xnxt, xcur

        if DEBUG_TAPS and l == 0:
            nc.sync.dma_start(io["dbg_x2"].rearrange("(k p) t -> p k t", p=P),
                              xcur[:])

        w2_ch = [wchunk(w2_r[:, ts(h, 4), :], l, f"w2{l}_{h}") for h in range(8)]

        # ---- FFN1: h^T = relu(W1^T x^T + b1)
        for m in range(FMT):
            ps1 = ps.tile([P, TL], f32, tag="mm", name=f"ps1{l}_{m}")
            for k in range(DT):
                nc.tensor.matmul(ps1[:], w1_ch[m // 4][:, k, ts(m % 4, P)],
                                 xb16[:, k, :], start=(k == 0), stop=(k == DT - 1))
            nc.scalar.activation(ht[:, m, :], ps1[:], AF.Relu,
                                 bias=b1_sb[:, m:m + 1])

        if DEBUG_TAPS and l == 0:
            nc.sync.dma_start(io["dbg_h"].rearrange("(k p) t -> p k t", p=P),
                              ht[:])

        # ---- FFN2 + residual -> y2 with fused BN partial stats
        st2 = spool.tile([P, 16], f32, tag="st", name=f"st2_{l}")
        sqs2 = spool.tile([P, TL], f32, tag="sqs", name=f"sq2_{l}")
        for m in range(DT):
            ps2 = ps.tile([P, TL], f32, tag="mm", name=f"ps2{l}_{m}")
            for k in range(FMT):
                nc.tensor.matmul(ps2[:], w2_ch[k // 4][:, k % 4, ts(m, P)],
                                 ht[:, k, :], start=(k == 0), stop=(k == FMT - 1))
            nc.vector.scalar_tensor_tensor(
                out=xnxt[:, m, :], in0=ps2[:], scalar=1.0, in1=xcur[:, m, :],
                op0=ALU.mult, op1=ALU.add, accum_out=st2[:, m:m + 1])
            if USE_TTR:
                nc.vector.tensor_tensor_reduce(
                    out=sqs2[:], in0=xnxt[:, m, :], in1=xnxt[:, m, :], scale=1.0,
                    scalar=0.0, op0=ALU.mult, op1=ALU.add,
                    accum_out=st2[:, DT + m:DT + m + 1])
            else:
                nc.scalar.activation(sqs2[:], xnxt[:, m, :], AF.Square,
                                     accum_out=st2[:, DT + m:DT + m + 1])

        if DEBUG_TAPS and l == 0:
            nc.sync.dma_start(io["dbg_y2"].rearrange("(k p) t -> p k t", p=P),
                              xnxt[:])

        sc2, sh2 = bn_reduce(f"f{l}", st2, g2_sb, be2_sb)
        for m in range(DT):
            nc.scalar.activation(xb16[:, m, :], xnxt[:, m, :], AF.Identity,
                                 bias=sh2[:, m:m + 1], scale=sc2[:, m:m + 1])
            nc.vector.tensor_scalar(out=xnxt[:, m, :], in0=xnxt[:, m, :],
                                    scalar1=sc2[:, m:m + 1],
                                    scalar2=sh2[:, m:m + 1],
                                    op0=ALU.mult, op1=ALU.add)
        xcur, xnxt = xnxt, xcur

    # ---------------- output x^T local slice
    dmae.dma_start(io["out"].rearrange("(k p) t -> p k t", p=P), xcur[:])
    st_.close()


# ================================================================ host side

def make_in_maps(inputs):
    import ml_dtypes
    bf = lambda a: np.ascontiguousarray(np.asarray(a, dtype=np.float32)).astype(
        ml_dtypes.bfloat16)
    f = lambda a: np.ascontiguousarray(np.asarray(a), dtype=np.float32)
    seq = np.asarray(inputs["sequence"]).reshape(-1).astype(np.int16)
    emb = bf(inputs["emb"])
    pesT = np.ascontiguousarray(f(inputs["pes"]).T)            # [D, S]
    wq, wk, wv = bf(inputs["Wq"]), bf(inputs["Wk"]), bf(inputs["Wv"])
    wo, w1, w2 = bf(inputs["Wo"]), bf(inputs["W1"]), bf(inputs["W2"])
    pt = lambda a, m: np.ascontiguousarray(
        f(a).reshape(L, m, P).transpose(0, 2, 1))   # [L, P, m] with ch = m*128+p
    bq, b1 = pt(inputs["bq"], DT), pt(inputs["b1"], FMT)
    g1, be1 = pt(inputs["g1"], DT), pt(inputs["be1"], DT)
    g2, be2 = pt(inputs["g2"], DT), pt(inputs["be2"], DT)

    in_maps = []
    for c in range(NC):
        loc = seq[c * TL:(c + 1) * TL]
        idx = np.ascontiguousarray(loc.reshape(TL // 16, 16).T)    # [16, TL/16]
        off = (c % 2) * TL
        in_maps.append({
            "emb": emb,
            "idx": idx,
            "pesT": np.ascontiguousarray(pesT[:, off:off + TL]),
            "wq": wq, "wk": wk, "wv": wv, "wo": wo, "w1": w1, "w2": w2,
            "bq": bq, "b1": b1,
            "g1": g1, "be1": be1, "g2": g2, "be2": be2,
        })
    return in_maps


_CACHE = {}


def _get_module():
    if "nc" not in _CACHE:
        _CACHE["nc"] = build_module()
    return _CACHE["nc"]


def kernel(**inputs):
    from concourse import bass_utils
    nc = _get_module()
    in_maps = make_in_maps(inputs)
    res = bass_utils.run_bass_kernel_spmd(nc, in_maps, list(range(NC)))
    full = np.concatenate(
        [np.asarray(res.results[c]["out"]) for c in range(NC)], axis=1)
    return np.ascontiguousarray(full.T).reshape(B, S, D).astype(np.float32)

